# revision 27
# baseline (speedup 1.0000x reference)
"""Multi-head attention (B=2, S=2048, H=1024, 16 heads) on 8 NeuronCores.

Tensor-parallel sharding: 2 heads per core.  Each core computes QKV for its
heads, full attention over the sequence for its heads, and a partial output
projection (its 128 rows of w_dense).  The host sums the 8 partial outputs
(the all-reduce) and adds the output-side bias terms.

v3 structure:
  * hs is transposed on the host and shipped as bf16 [HID, SEQ]; QKV
    weights in bf16.  Attention math stays f32r.
  * K-bias dropped on device (softmax-invariant, exact); V/dense biases
    commute to the host.
  * ctx is normalized (divided by the softmax row sums) BEFORE the output
    projection: the per-q reciprocal row is broadcast across partitions by
    the otherwise-idle GpSimd engine, so both heads' dense contributions
    collapse into a single matmul per output tile (half the dense PE work,
    and the tiny denominator-transpose matmuls disappear).
  * The dense matmuls of block N-1 are interleaved into block N's
    attention loop (one per two kt-steps): the loop is rate-limited by the
    Scalar engine's EXP, so the PE slack absorbs the dense work.

Layout notes (per core), all PE matmuls in plain 128x128 mode:
  hsw  [128, hid/128, 512]  bf16 window of host-pretransposed hs.
  QTz/KTz [128, h, seq] q/k transposed per head, zero-padded to a full
                        128-partition contraction (rows 64-127 = 0).
  Vn  [128, 32, 2, 66]  v natural: partition = seq within 128-chunk,
                        [chunk, head, dim]; col 64 is 1.0 so the P@V
                        matmul also emits the softmax denominators.
  PT  [128, RING, 1024] exp(scores) ring: partition = k within chunk.
  ctxT [128, seq]       context transposed, head 0 rows 0-63 and head 1
                        rows 64-127, so one dense matmul contracts both
                        heads against full-width w_dense slices.
"""

import os
import sys
import types

sys.path.insert(0, "/opt/trn_rl_repo")

import numpy as np

try:
    import ml_dtypes

    BF16_NP = ml_dtypes.bfloat16
except ImportError:  # pragma: no cover
    BF16_NP = None


def _install_ntff_shim():
    """The trimmed container image lacks ``antenv.axon_hooks``, which
    ``run_bass_kernel_spmd(trace=True)`` needs to capture NTFF profiles
    under axon.  Recreate it from the boot helper + the injected .so."""
    if "antenv.axon_hooks" in sys.modules:
        return
    try:
        from trn_agent_boot.trn_boot import _ntff_profile_via_ctypes
        so = "/opt/axon/libaxon_pjrt.so"
        if not os.path.exists(so):
            return
        hook = _ntff_profile_via_ctypes(so)
        mod = types.ModuleType("antenv.axon_hooks")
        mod.get_axon_ntff_profile_hook = lambda: hook
        mod.set_axon_ntff_profile_hook = lambda h: None
        sys.modules["antenv.axon_hooks"] = mod
    except Exception:
        pass


_install_ntff_shim()

import concourse.bass as bass
import concourse.mybir as mybir
import concourse.tile as tile
from concourse import bacc
from concourse.bass_utils import run_bass_kernel_spmd
from concourse.masks import make_identity

F32 = mybir.dt.float32
F32R = mybir.dt.float32r
BF16 = mybir.dt.bfloat16
EXP = mybir.ActivationFunctionType.Exp

B, S, HID = 2, 2048, 1024
HEADS, D = 16, 64
SEQ = B * S                      # 4096 flattened rows
NCORES = 8
HPC = HEADS // NCORES            # heads per core = 2
CW = HPC * D                     # per-core width = 128
NHB = HID // 128                 # hidden 128-chunks = 8
WSEQ = 512                       # seq window for QKV
NWIN = SEQ // WSEQ               # 8
QW = 1024                        # q window in attention
NKT = S // 128                   # k chunks per batch = 16
NCH = SEQ // 128                 # global 128-row chunks = 32
RING = 5


def build_nc():
    nc = bacc.Bacc("TRN2", target_bir_lowering=False, debug=False,
                   num_devices=NCORES)

    hsT = nc.dram_tensor("hsT", [HID, SEQ], BF16, kind="ExternalInput")
    w3 = nc.dram_tensor("w3", [HID, 3 * CW], BF16, kind="ExternalInput")
    bq = nc.dram_tensor("bq", [CW, 1], F32, kind="ExternalInput")
    wd = nc.dram_tensor("wd", [CW, HID], F32, kind="ExternalInput")
    out = nc.dram_tensor("out", [SEQ, HID], BF16, kind="ExternalOutput")

    with tile.TileContext(nc) as tc:
        with (
            tc.tile_pool(name="persist", bufs=1) as pp,
            tc.tile_pool(name="pt", bufs=1) as ptp,
            tc.tile_pool(name="hsload", bufs=2) as hlp,
        ):
            # first hs window DMA goes out before the weight loads so the
            # PE can start as early as possible; its first half lands even
            # sooner so the first matmul chain can begin
            hsTd = hsT.ap().rearrange("(c p) s -> p c s", p=128)
            hsw0 = hlp.tile([128, NHB, WSEQ], BF16)
            nc.sync.dma_start(hsw0[:], hsTd[:, :, 0:WSEQ])

            ident = pp.tile([128, 128], F32)
            make_identity(nc, ident[:])
            identr_t = pp.tile([128, 128], F32R)
            nc.vector.tensor_copy(identr_t[:], ident[:])
            identr = identr_t[:]

            w3_sb = pp.tile([128, NHB, 3 * CW], BF16)
            nc.gpsimd.dma_start(
                w3_sb[:], w3.ap().rearrange("(c p) m -> p c m", p=128))
            bq_sb = pp.tile([CW, 1], F32)
            nc.gpsimd.dma_start(bq_sb[:], bq[:])

            # Per-head q/k operands are zero-padded to a full 128-partition
            # contraction: QTz/KTz [:, h, :] rows 0-63 = head h, rows
            # 64-127 = 0.  ctxT packs head 0 in rows 0-63 and head 1 in
            # rows 64-127 (no padding needed).
            QTz = pp.tile([128, HPC, SEQ], F32R)
            KTz = pp.tile([128, HPC, SEQ], F32R)
            Vn = pp.tile([128, NCH, HPC, 66], F32R)
            ctxT = pp.tile([128, SEQ], F32R)
            den2 = pp.tile([1, HPC, QW], F32R)   # partition-0 rowsum rows
            bcden = pp.tile([128, HPC, QW], F32)  # broadcast 1/rowsums
            ones_row = pp.tile([1, 128], F32R)    # bcast matmul weights
            PT = ptp.tile([128, RING, QW], F32R)

            # ones column used by the P@V matmul to emit row sums
            ones_st = pp.tile([128, NCH * HPC], F32)
            nc.vector.memset(ones_st[:], 1.0)
            nc.vector.tensor_copy(
                Vn[:, :, :, 64:65],
                ones_st[:].rearrange("p (c h) -> p c h", c=NCH)
                .rearrange("p c h -> p c h ()"))
            ones_f = pp.tile([1, 128], F32)
            nc.vector.memset(ones_f[:], 1.0)
            nc.vector.tensor_copy(ones_row[:], ones_f[:])

            # zero-fill the padded q/k halves on the (early-idle) vector
            # engine (f32r memset is rejected by the ISA checker, so bounce
            # through an f32 staging tile kept in the persistent pool)
            zs = pp.tile([D, SEQ // 4], F32)
            nc.vector.memset(zs[:], 0.0)
            for z0 in range(0, SEQ, SEQ // 4):
                zl = slice(z0, z0 + SEQ // 4)
                for h in range(HPC):
                    nc.vector.tensor_copy(QTz[D:128, h, zl], zs[:])
                    nc.vector.tensor_copy(KTz[D:128, h, zl], zs[:])

            # dense weights are first needed ~90us in; keep their DMA out
            # of the startup critical path
            wd_sb = pp.tile([CW, HID], F32R)
            nc.gpsimd.dma_start(wd_sb[:], wd.ap().bitcast(F32R))

            # ---------------- phase 1: QKV projections off pre-transposed
            # hs windows (streamed from DRAM in bf16) ----------------------
            with (
                tc.tile_pool(name="vtw", bufs=2) as vwp,
                tc.tile_pool(name="ps_qkv", bufs=3,
                             space=bass.MemorySpace.PSUM) as pqk,
                tc.tile_pool(name="ps_tr", bufs=2,
                             space=bass.MemorySpace.PSUM) as ptr,
            ):
                for w in range(NWIN):
                    r0 = w * WSEQ
                    wsl = slice(r0, r0 + WSEQ)
                    if w == 0:
                        hsw = hsw0
                    else:
                        hsw = hlp.tile([128, NHB, WSEQ], BF16)
                        nc.sync.dma_start(hsw[:], hsTd[:, :, wsl])
                    for tgt in range(3):
                        ps = pqk.tile([128, WSEQ], F32, tag="qkv")
                        wslc = w3_sb[:, :, tgt * CW:(tgt + 1) * CW]
                        for hb in range(NHB):
                            nc.tensor.matmul(
                                ps[:], wslc[:, hb, :], hsw[:, hb, :],
                                start=(hb == 0), stop=(hb == NHB - 1))
                        if tgt == 0:
                            for h in range(HPC):
                                nc.vector.tensor_scalar_add(
                                    QTz[0:D, h, wsl],
                                    ps[h * D:(h + 1) * D, :],
                                    bq_sb[h * D:(h + 1) * D, 0:1])
                        elif tgt == 1:
                            # k-bias shifts every logit of a q-row equally;
                            # softmax is invariant, so it is dropped (exact)
                            for h in range(HPC):
                                nc.vector.tensor_copy(
                                    KTz[0:D, h, wsl],
                                    ps[h * D:(h + 1) * D, :])
                        else:
                            vtw = vwp.tile([128, WSEQ], F32R)
                            nc.vector.tensor_copy(vtw[:], ps[:])
                            vps = ptr.tile([128, WSEQ], F32, tag="vtr")
                            for sb2 in range(WSEQ // 128):
                                nc.tensor.transpose(
                                    vps[:, sb2 * 128:(sb2 + 1) * 128]
                                    .bitcast(F32R),
                                    vtw[:, sb2 * 128:(sb2 + 1) * 128],
                                    identr)
                            ch0 = r0 // 128
                            nc.vector.tensor_copy(
                                Vn[:, ch0:ch0 + 4, :, 0:64],
                                vps[:].rearrange("p (c h d) -> p c h d",
                                                 c=4, h=HPC))

            # ---------------- phase 2: attention + output projection -----
            # The dense matmuls of the previous (b, qw) block are emitted
            # into the attention loop (one per two kt-steps): the loop is
            # scalar(EXP)-bound, so the PE slack absorbs them.
            with (
                tc.tile_pool(name="ps_st", bufs=2,
                             space=bass.MemorySpace.PSUM) as pst,
                tc.tile_pool(name="ps_pv", bufs=1,
                             space=bass.MemorySpace.PSUM) as ppv,
                tc.tile_pool(name="ps_dn", bufs=2,
                             space=bass.MemorySpace.PSUM) as pdn,
                tc.tile_pool(name="outst", bufs=4) as osp,
            ):
                def dense_steps(qbase, evict_split=False):
                    """Generator: 16 micro-steps for one block's (merged
                    two-head) dense output projection.  In the final flush
                    (evict_split) alternate evictions between Scalar and
                    Vector so the tail drains twice as fast."""
                    i = 0
                    for stl in range(QW // 128):
                        st = qbase // 128 + stl
                        ssl = slice(st * 128, (st + 1) * 128)
                        for nt in range(HID // 512):
                            nsl = slice(nt * 512, (nt + 1) * 512)
                            psd = pdn.tile([128, 512], F32, tag="dn")
                            nc.tensor.matmul(
                                psd[:], ctxT[:, ssl], wd_sb[:, nsl],
                                start=True, stop=True)
                            ob = osp.tile([128, 512], BF16)
                            if evict_split and i % 2 == 0:
                                nc.scalar.copy(ob[:], psd[:])
                            else:
                                nc.vector.tensor_copy(ob[:], psd[:])
                            nc.sync.dma_start(out[ssl, nsl], ob[:])
                            i += 1
                            yield

                def norm_chain(qbase, h):
                    """Normalize head h's ctx block: broadcast its rowsum
                    row to all partitions with a 1-row PE matmul, then a
                    wide reciprocal + one in-place multiply."""
                    for half in range(QW // 512):
                        hsl = slice(half * 512, (half + 1) * 512)
                        bcp = pdn.tile([128, 512], F32, tag="dn")
                        nc.tensor.matmul(
                            bcp[:], ones_row[:], den2[0:1, h, hsl],
                            start=True, stop=True)
                        nc.vector.reciprocal_approx_fast(
                            bcden[:, h, hsl], bcp[:])
                    nc.vector.tensor_mul(
                        ctxT[h * D:(h + 1) * D, qbase:qbase + QW],
                        ctxT[h * D:(h + 1) * D, qbase:qbase + QW]
                        .bitcast(F32),
                        bcden[h * D:(h + 1) * D, h, :])

                pending = None   # dense generator of the previous block
                norm_todo = []   # deferred per-head normalize chains
                ktg = 0          # global kt counter -> PT ring slot, so
                                 # loop boundaries don't collide on a slot
                for b in range(B):
                    for qw in range(S // QW):
                        qbase = b * S + qw * QW
                        for hh in range(HPC):
                            pva = ppv.tile([D + 1, 512], F32, tag="pva")
                            pvb = ppv.tile([D + 1, 512], F32, tag="pvb")
                            for kt in range(NKT):
                                ch = b * NKT + kt
                                ksl = slice(b * S + kt * 128,
                                            b * S + (kt + 1) * 128)
                                rg = ktg % RING
                                ktg += 1
                                stp = pst.tile([128, QW], F32, tag="st")
                                for qh in range(QW // 512):
                                    sl = slice(qh * 512, (qh + 1) * 512)
                                    nc.tensor.matmul(
                                        stp[:, sl], KTz[:, hh, ksl],
                                        QTz[:, hh,
                                            qbase + qh * 512:
                                            qbase + (qh + 1) * 512],
                                        start=True, stop=True)
                                nc.scalar.activation(
                                    PT[:, rg, :], stp[:], EXP, scale=0.125)
                                for qh, pvh in ((0, pva), (1, pvb)):
                                    sl = slice(qh * 512, (qh + 1) * 512)
                                    nc.tensor.matmul(
                                        pvh[:], Vn[:, ch, hh, 0:65],
                                        PT[:, rg, sl],
                                        start=(kt == 0),
                                        stop=(kt == NKT - 1))
                                # deferred work rides the scalar-bound loop:
                                # normalize chains at kt==1, one dense step
                                # of the previous block per later kt
                                if kt == 1 and norm_todo:
                                    norm_chain(*norm_todo.pop(0))
                                elif pending is not None and (
                                        (kt % 2 == 1 and kt >= 3)
                                        or (hh == 1 and kt % 2 == 0
                                            and kt >= 12)):
                                    next(pending, None)
                            # denominator rows go out first so the
                            # broadcast matmul's input is ready early;
                            # ctx follows
                            for qh, pvh in ((0, pva), (1, pvb)):
                                dsl = slice(qh * 512, (qh + 1) * 512)
                                nc.vector.tensor_copy(
                                    den2[0:1, hh, dsl], pvh[D:D + 1, :])
                            for qh, pvh in ((0, pva), (1, pvb)):
                                s2 = slice(qbase + qh * 512,
                                           qbase + (qh + 1) * 512)
                                nc.vector.tensor_copy(
                                    ctxT[hh * D:(hh + 1) * D, s2],
                                    pvh[0:D, :])
                            norm_todo.append((qbase, hh))
                        # drain dense leftovers of the previous block
                        if pending is not None:
                            for _ in pending:
                                pass
                        last = (b == B - 1 and qw == S // QW - 1)
                        pending = dense_steps(qbase, evict_split=last)
                # flush: last head's normalize + last block's dense
                while norm_todo:
                    norm_chain(*norm_todo.pop(0))
                for _ in pending:
                    pass

    nc.compile()
    return nc


_NC_CACHE = None


def get_nc():
    global _NC_CACHE
    if _NC_CACHE is None:
        _NC_CACHE = build_nc()
    return _NC_CACHE


def make_in_maps(hidden_states, w_qkv, b_qkv, w_dense):
    hs = np.asarray(hidden_states, dtype=np.float32).reshape(SEQ, HID)
    hsT = np.ascontiguousarray(hs.T).astype(BF16_NP)
    w_qkv = np.asarray(w_qkv, dtype=np.float32)
    b_qkv = np.asarray(b_qkv, dtype=np.float32)
    w_dense = np.asarray(w_dense, dtype=np.float32)
    # Reference layout: qkv.reshape(B, S, HEADS, 3*D) split on the last
    # axis, i.e. w_qkv columns are per-head [q_h | k_h | v_h] blocks of D.
    wq_cols = np.concatenate(
        [np.arange(h * 3 * D, h * 3 * D + D) for h in range(HEADS)])
    wk_cols = wq_cols + D
    wv_cols = wq_cols + 2 * D
    in_maps = []
    for c in range(NCORES):
        c0 = c * CW
        sel = slice(c0, c0 + CW)
        w3 = np.concatenate(
            [w_qkv[:, wq_cols[sel]], w_qkv[:, wk_cols[sel]],
             w_qkv[:, wv_cols[sel]]], axis=1).astype(BF16_NP)
        in_maps.append({
            "hsT": hsT,
            "w3": np.ascontiguousarray(w3),
            "bq": np.ascontiguousarray(b_qkv[wq_cols[sel]].reshape(CW, 1)),
            "wd": np.ascontiguousarray(w_dense[sel, :]),
        })
    return in_maps


def run(hidden_states, w_qkv, b_qkv, w_dense, b_dense, trace=False):
    nc = get_nc()
    in_maps = make_in_maps(hidden_states, w_qkv, b_qkv, w_dense)
    res = run_bass_kernel_spmd(nc, in_maps, core_ids=list(range(NCORES)),
                               trace=trace)
    acc = res.results[0]["out"].astype(np.float32)
    for c in range(1, NCORES):
        acc = acc + res.results[c]["out"]
    # bias terms that commute to the end: v-bias through dense, dense bias
    b_qkv = np.asarray(b_qkv, dtype=np.float32)
    b_v = np.concatenate(
        [b_qkv[h * 3 * D + 2 * D:h * 3 * D + 3 * D] for h in range(HEADS)])
    acc = acc + (b_v @ np.asarray(w_dense, dtype=np.float32)
                 + np.asarray(b_dense, dtype=np.float32))
    return acc.reshape(B, S, HID).astype(np.float32), res


def kernel(hidden_states, w_qkv, b_qkv, w_dense, b_dense):
    out, _ = run(hidden_states, w_qkv, b_qkv, w_dense, b_dense,
                 trace=bool(os.environ.get("BASS_TRACE")))
    return out


# revision 28
# speedup vs baseline: 1.0194x; 1.0194x over previous
"""Multi-head attention (B=2, S=2048, H=1024, 16 heads) on 8 NeuronCores.

Tensor-parallel sharding: 2 heads per core.  Each core computes QKV for its
heads, full attention over the sequence for its heads, and a partial output
projection (its 128 rows of w_dense).  The host sums the 8 partial outputs
(the all-reduce) and adds the output-side bias terms.

v3 structure:
  * hs is transposed on the host and shipped as bf16 [HID, SEQ]; QKV
    weights in bf16.  Attention math stays f32r.
  * K-bias dropped on device (softmax-invariant, exact); V/dense biases
    commute to the host.
  * ctx is normalized (divided by the softmax row sums) BEFORE the output
    projection: the per-q reciprocal row is broadcast across partitions by
    the otherwise-idle GpSimd engine, so both heads' dense contributions
    collapse into a single matmul per output tile (half the dense PE work,
    and the tiny denominator-transpose matmuls disappear).
  * The dense matmuls of block N-1 are interleaved into block N's
    attention loop (one per two kt-steps): the loop is rate-limited by the
    Scalar engine's EXP, so the PE slack absorbs the dense work.

Layout notes (per core), all PE matmuls in plain 128x128 mode:
  hsw  [128, hid/128, 512]  bf16 window of host-pretransposed hs.
  QTz/KTz [128, h, seq] q/k transposed per head, zero-padded to a full
                        128-partition contraction (rows 64-127 = 0).
  Vn  [128, 32, 2, 66]  v natural: partition = seq within 128-chunk,
                        [chunk, head, dim]; col 64 is 1.0 so the P@V
                        matmul also emits the softmax denominators.
  PT  [128, RING, 1024] exp(scores) ring: partition = k within chunk.
  ctxT [128, seq]       context transposed, head 0 rows 0-63 and head 1
                        rows 64-127, so one dense matmul contracts both
                        heads against full-width w_dense slices.
"""

import os
import sys
import types

sys.path.insert(0, "/opt/trn_rl_repo")

import numpy as np

try:
    import ml_dtypes

    BF16_NP = ml_dtypes.bfloat16
except ImportError:  # pragma: no cover
    BF16_NP = None


def _install_ntff_shim():
    """The trimmed container image lacks ``antenv.axon_hooks``, which
    ``run_bass_kernel_spmd(trace=True)`` needs to capture NTFF profiles
    under axon.  Recreate it from the boot helper + the injected .so."""
    if "antenv.axon_hooks" in sys.modules:
        return
    try:
        from trn_agent_boot.trn_boot import _ntff_profile_via_ctypes
        so = "/opt/axon/libaxon_pjrt.so"
        if not os.path.exists(so):
            return
        hook = _ntff_profile_via_ctypes(so)
        mod = types.ModuleType("antenv.axon_hooks")
        mod.get_axon_ntff_profile_hook = lambda: hook
        mod.set_axon_ntff_profile_hook = lambda h: None
        sys.modules["antenv.axon_hooks"] = mod
    except Exception:
        pass


_install_ntff_shim()

import concourse.bass as bass
import concourse.mybir as mybir
import concourse.tile as tile
from concourse import bacc
from concourse.bass_utils import run_bass_kernel_spmd
from concourse.masks import make_identity

F32 = mybir.dt.float32
F32R = mybir.dt.float32r
BF16 = mybir.dt.bfloat16
EXP = mybir.ActivationFunctionType.Exp

B, S, HID = 2, 2048, 1024
HEADS, D = 16, 64
SEQ = B * S                      # 4096 flattened rows
NCORES = 8
HPC = HEADS // NCORES            # heads per core = 2
CW = HPC * D                     # per-core width = 128
NHB = HID // 128                 # hidden 128-chunks = 8
WSEQ = 512                       # seq window for QKV
NWIN = SEQ // WSEQ               # 8
QW = 1024                        # q window in attention
NKT = S // 128                   # k chunks per batch = 16
NCH = SEQ // 128                 # global 128-row chunks = 32
RING = 5


def build_nc():
    nc = bacc.Bacc("TRN2", target_bir_lowering=False, debug=False,
                   num_devices=NCORES)

    hsT = nc.dram_tensor("hsT", [HID, SEQ], BF16, kind="ExternalInput")
    w3 = nc.dram_tensor("w3", [HID, 3 * CW], BF16, kind="ExternalInput")
    bq = nc.dram_tensor("bq", [CW, 1], F32, kind="ExternalInput")
    wd = nc.dram_tensor("wd", [CW, HID], F32, kind="ExternalInput")
    out = nc.dram_tensor("out", [SEQ, HID], BF16, kind="ExternalOutput")

    with tile.TileContext(nc) as tc:
        with (
            tc.tile_pool(name="persist", bufs=1) as pp,
            tc.tile_pool(name="pt", bufs=1) as ptp,
            tc.tile_pool(name="hsload", bufs=2) as hlp,
        ):
            # first hs window DMA goes out before the weight loads so the
            # PE can start as early as possible; its first half lands even
            # sooner so the first matmul chain can begin
            hsTd = hsT.ap().rearrange("(c p) s -> p c s", p=128)
            hsw0 = hlp.tile([128, NHB, WSEQ], BF16)
            nc.sync.dma_start(hsw0[:], hsTd[:, :, 0:WSEQ])

            ident = pp.tile([128, 128], F32)
            make_identity(nc, ident[:])
            identr_t = pp.tile([128, 128], F32R)
            nc.vector.tensor_copy(identr_t[:], ident[:])
            identr = identr_t[:]

            w3_sb = pp.tile([128, NHB, 3 * CW], BF16)
            nc.gpsimd.dma_start(
                w3_sb[:], w3.ap().rearrange("(c p) m -> p c m", p=128))
            bq_sb = pp.tile([CW, 1], F32)
            nc.gpsimd.dma_start(bq_sb[:], bq[:])

            # Per-head q/k operands are zero-padded to a full 128-partition
            # contraction: QTz/KTz [:, h, :] rows 0-63 = head h, rows
            # 64-127 = 0.  ctxT packs head 0 in rows 0-63 and head 1 in
            # rows 64-127 (no padding needed).
            QTz = pp.tile([128, HPC, SEQ], F32R)
            KTz = pp.tile([128, HPC, SEQ], F32R)
            Vn = pp.tile([128, NCH, HPC, 66], F32R)
            ctxT = pp.tile([128, SEQ], F32R)
            den2 = pp.tile([1, HPC, QW], F32R)   # partition-0 rowsum rows
            bcden = pp.tile([128, HPC, QW], F32)  # broadcast 1/rowsums
            ones_row = pp.tile([1, 128], F32R)    # bcast matmul weights
            PT = ptp.tile([128, RING, QW], F32R)

            # ones column used by the P@V matmul to emit row sums
            ones_st = pp.tile([128, NCH * HPC], F32)
            nc.vector.memset(ones_st[:], 1.0)
            nc.vector.tensor_copy(
                Vn[:, :, :, 64:65],
                ones_st[:].rearrange("p (c h) -> p c h", c=NCH)
                .rearrange("p c h -> p c h ()"))
            ones_f = pp.tile([1, 128], F32)
            nc.vector.memset(ones_f[:], 1.0)
            nc.vector.tensor_copy(ones_row[:], ones_f[:])

            # zero-fill the padded q/k halves on the (early-idle) vector
            # engine (f32r memset is rejected by the ISA checker, so bounce
            # through an f32 staging tile kept in the persistent pool)
            zs = pp.tile([D, SEQ // 4], F32)
            nc.vector.memset(zs[:], 0.0)
            for z0 in range(0, SEQ, SEQ // 4):
                zl = slice(z0, z0 + SEQ // 4)
                for h in range(HPC):
                    nc.vector.tensor_copy(QTz[D:128, h, zl], zs[:])
                    nc.vector.tensor_copy(KTz[D:128, h, zl], zs[:])

            # dense weights are first needed ~90us in; keep their DMA out
            # of the startup critical path
            wd_sb = pp.tile([CW, HID], F32R)
            nc.gpsimd.dma_start(wd_sb[:], wd.ap().bitcast(F32R))

            # ---------------- phase 1: QKV projections off pre-transposed
            # hs windows (streamed from DRAM in bf16) ----------------------
            with (
                tc.tile_pool(name="vtw", bufs=2) as vwp,
                tc.tile_pool(name="ps_qkv", bufs=3,
                             space=bass.MemorySpace.PSUM) as pqk,
                tc.tile_pool(name="ps_tr", bufs=2,
                             space=bass.MemorySpace.PSUM) as ptr,
            ):
                for w in range(NWIN):
                    r0 = w * WSEQ
                    wsl = slice(r0, r0 + WSEQ)
                    if w == 0:
                        hsw = hsw0
                    else:
                        hsw = hlp.tile([128, NHB, WSEQ], BF16)
                        nc.sync.dma_start(hsw[:], hsTd[:, :, wsl])
                    for tgt in range(3):
                        ps = pqk.tile([128, WSEQ], F32, tag="qkv")
                        wslc = w3_sb[:, :, tgt * CW:(tgt + 1) * CW]
                        for hb in range(NHB):
                            nc.tensor.matmul(
                                ps[:], wslc[:, hb, :], hsw[:, hb, :],
                                start=(hb == 0), stop=(hb == NHB - 1))
                        if tgt == 0:
                            for h in range(HPC):
                                nc.vector.tensor_scalar_add(
                                    QTz[0:D, h, wsl],
                                    ps[h * D:(h + 1) * D, :],
                                    bq_sb[h * D:(h + 1) * D, 0:1])
                        elif tgt == 1:
                            # k-bias shifts every logit of a q-row equally;
                            # softmax is invariant, so it is dropped (exact)
                            for h in range(HPC):
                                nc.vector.tensor_copy(
                                    KTz[0:D, h, wsl],
                                    ps[h * D:(h + 1) * D, :])
                        else:
                            vtw = vwp.tile([128, WSEQ], F32R)
                            nc.vector.tensor_copy(vtw[:], ps[:])
                            vps = ptr.tile([128, WSEQ], F32, tag="vtr")
                            for sb2 in range(WSEQ // 128):
                                nc.tensor.transpose(
                                    vps[:, sb2 * 128:(sb2 + 1) * 128]
                                    .bitcast(F32R),
                                    vtw[:, sb2 * 128:(sb2 + 1) * 128],
                                    identr)
                            ch0 = r0 // 128
                            nc.vector.tensor_copy(
                                Vn[:, ch0:ch0 + 4, :, 0:64],
                                vps[:].rearrange("p (c h d) -> p c h d",
                                                 c=4, h=HPC))

            # ---------------- phase 2: attention + output projection -----
            # The dense matmuls of the previous (b, qw) block are emitted
            # into the attention loop (one per two kt-steps): the loop is
            # scalar(EXP)-bound, so the PE slack absorbs them.
            with (
                tc.tile_pool(name="ps_st", bufs=2,
                             space=bass.MemorySpace.PSUM) as pst,
                tc.tile_pool(name="ps_pv", bufs=1,
                             space=bass.MemorySpace.PSUM) as ppv,
                tc.tile_pool(name="ps_dn", bufs=2,
                             space=bass.MemorySpace.PSUM) as pdn,
                tc.tile_pool(name="outst", bufs=6) as osp,
            ):
                def dense_steps(qbase, evict_split=False):
                    """Generator: 16 micro-steps for one block's (merged
                    two-head) dense output projection.  In the final flush
                    (evict_split) alternate evictions between Scalar and
                    Vector so the tail drains twice as fast."""
                    i = 0
                    for stl in range(QW // 128):
                        st = qbase // 128 + stl
                        ssl = slice(st * 128, (st + 1) * 128)
                        for nt in range(HID // 512):
                            nsl = slice(nt * 512, (nt + 1) * 512)
                            psd = pdn.tile([128, 512], F32, tag="dn")
                            nc.tensor.matmul(
                                psd[:], ctxT[:, ssl], wd_sb[:, nsl],
                                start=True, stop=True)
                            ob = osp.tile([128, 512], BF16)
                            if evict_split and i % 2 == 0:
                                nc.scalar.copy(ob[:], psd[:])
                            else:
                                nc.vector.tensor_copy(ob[:], psd[:])
                            nc.sync.dma_start(out[ssl, nsl], ob[:])
                            i += 1
                            yield

                def norm_chain(qbase, h):
                    """Normalize head h's ctx block: broadcast its rowsum
                    row to all partitions with a 1-row PE matmul, then a
                    wide reciprocal + one in-place multiply."""
                    for half in range(QW // 512):
                        hsl = slice(half * 512, (half + 1) * 512)
                        bcp = pdn.tile([128, 512], F32, tag="dn")
                        nc.tensor.matmul(
                            bcp[:], ones_row[:], den2[0:1, h, hsl],
                            start=True, stop=True)
                        nc.vector.reciprocal_approx_fast(
                            bcden[:, h, hsl], bcp[:])
                    nc.vector.tensor_mul(
                        ctxT[h * D:(h + 1) * D, qbase:qbase + QW],
                        ctxT[h * D:(h + 1) * D, qbase:qbase + QW]
                        .bitcast(F32),
                        bcden[h * D:(h + 1) * D, h, :])

                pending = None   # dense generator of the previous block
                norm_todo = []   # deferred per-head normalize chains
                ktg = 0          # global kt counter -> PT ring slot, so
                                 # loop boundaries don't collide on a slot
                for b in range(B):
                    for qw in range(S // QW):
                        qbase = b * S + qw * QW
                        for hh in range(HPC):
                            pva = ppv.tile([D + 1, 512], F32, tag="pva")
                            pvb = ppv.tile([D + 1, 512], F32, tag="pvb")
                            for kt in range(NKT):
                                ch = b * NKT + kt
                                ksl = slice(b * S + kt * 128,
                                            b * S + (kt + 1) * 128)
                                rg = ktg % RING
                                ktg += 1
                                stp = pst.tile([128, QW], F32, tag="st")
                                for qh in range(QW // 512):
                                    sl = slice(qh * 512, (qh + 1) * 512)
                                    nc.tensor.matmul(
                                        stp[:, sl], KTz[:, hh, ksl],
                                        QTz[:, hh,
                                            qbase + qh * 512:
                                            qbase + (qh + 1) * 512],
                                        start=True, stop=True)
                                nc.scalar.activation(
                                    PT[:, rg, :], stp[:], EXP, scale=0.125)
                                for qh, pvh in ((0, pva), (1, pvb)):
                                    sl = slice(qh * 512, (qh + 1) * 512)
                                    nc.tensor.matmul(
                                        pvh[:], Vn[:, ch, hh, 0:65],
                                        PT[:, rg, sl],
                                        start=(kt == 0),
                                        stop=(kt == NKT - 1))
                                # deferred work rides the scalar-bound loop:
                                # normalize chains at kt==1, one dense step
                                # of the previous block per later kt
                                if kt == 0 and norm_todo:
                                    norm_chain(*norm_todo.pop(0))
                                elif pending is not None and (
                                        (kt % 2 == 1
                                         and kt >= (5 if hh == 0 else 3))
                                        or (hh == 1 and kt % 2 == 0
                                            and kt >= 12)):
                                    next(pending, None)
                            # denominator rows go out first so the
                            # broadcast matmul's input is ready early;
                            # ctx follows
                            for qh, pvh in ((0, pva), (1, pvb)):
                                dsl = slice(qh * 512, (qh + 1) * 512)
                                nc.scalar.copy(
                                    den2[0:1, hh, dsl], pvh[D:D + 1, :])
                            for qh, pvh in ((0, pva), (1, pvb)):
                                s2 = slice(qbase + qh * 512,
                                           qbase + (qh + 1) * 512)
                                nc.vector.tensor_copy(
                                    ctxT[hh * D:(hh + 1) * D, s2],
                                    pvh[0:D, :])
                            norm_todo.append((qbase, hh))
                        # drain dense leftovers of the previous block
                        if pending is not None:
                            for _ in pending:
                                pass
                        last = (b == B - 1 and qw == S // QW - 1)
                        pending = dense_steps(qbase, evict_split=last)
                # flush: last head's normalize + last block's dense
                while norm_todo:
                    norm_chain(*norm_todo.pop(0))
                for _ in pending:
                    pass

    nc.compile()
    return nc


_NC_CACHE = None


def get_nc():
    global _NC_CACHE
    if _NC_CACHE is None:
        _NC_CACHE = build_nc()
    return _NC_CACHE


def make_in_maps(hidden_states, w_qkv, b_qkv, w_dense):
    hs = np.asarray(hidden_states, dtype=np.float32).reshape(SEQ, HID)
    hsT = np.ascontiguousarray(hs.T).astype(BF16_NP)
    w_qkv = np.asarray(w_qkv, dtype=np.float32)
    b_qkv = np.asarray(b_qkv, dtype=np.float32)
    w_dense = np.asarray(w_dense, dtype=np.float32)
    # Reference layout: qkv.reshape(B, S, HEADS, 3*D) split on the last
    # axis, i.e. w_qkv columns are per-head [q_h | k_h | v_h] blocks of D.
    wq_cols = np.concatenate(
        [np.arange(h * 3 * D, h * 3 * D + D) for h in range(HEADS)])
    wk_cols = wq_cols + D
    wv_cols = wq_cols + 2 * D
    in_maps = []
    for c in range(NCORES):
        c0 = c * CW
        sel = slice(c0, c0 + CW)
        w3 = np.concatenate(
            [w_qkv[:, wq_cols[sel]], w_qkv[:, wk_cols[sel]],
             w_qkv[:, wv_cols[sel]]], axis=1).astype(BF16_NP)
        in_maps.append({
            "hsT": hsT,
            "w3": np.ascontiguousarray(w3),
            "bq": np.ascontiguousarray(b_qkv[wq_cols[sel]].reshape(CW, 1)),
            "wd": np.ascontiguousarray(w_dense[sel, :]),
        })
    return in_maps


def run(hidden_states, w_qkv, b_qkv, w_dense, b_dense, trace=False):
    nc = get_nc()
    in_maps = make_in_maps(hidden_states, w_qkv, b_qkv, w_dense)
    res = run_bass_kernel_spmd(nc, in_maps, core_ids=list(range(NCORES)),
                               trace=trace)
    acc = res.results[0]["out"].astype(np.float32)
    for c in range(1, NCORES):
        acc = acc + res.results[c]["out"]
    # bias terms that commute to the end: v-bias through dense, dense bias
    b_qkv = np.asarray(b_qkv, dtype=np.float32)
    b_v = np.concatenate(
        [b_qkv[h * 3 * D + 2 * D:h * 3 * D + 3 * D] for h in range(HEADS)])
    acc = acc + (b_v @ np.asarray(w_dense, dtype=np.float32)
                 + np.asarray(b_dense, dtype=np.float32))
    return acc.reshape(B, S, HID).astype(np.float32), res


def kernel(hidden_states, w_qkv, b_qkv, w_dense, b_dense):
    out, _ = run(hidden_states, w_qkv, b_qkv, w_dense, b_dense,
                 trace=bool(os.environ.get("BASS_TRACE")))
    return out
